# revision 3
# baseline (speedup 1.0000x reference)
"""Batched greedy GRU decoder on 8 Trainium2 NeuronCores.

Strategy: tensor-parallel over the vocabulary. W_proj [32000,512] fp32 (65.5MB)
cannot fit in one core's 28MB SBUF; an 8-way shard (4096 rows/core) is kept
SBUF-resident across all 64 decode steps as a bf16 hi/lo SPLIT PAIR (8MB):
W = W_hi + W_lo with each half bf16. The logit scan runs as THREE 1-cycle/row
bf16 passes (W_hi*h_hi + W_hi*h_lo + W_lo*h_hi, fp32 PSUM accumulate), giving
~2^-17 relative precision -- verified offline to reproduce the fp32 argmax
trajectory token-exactly with 75x margin -- at 3/4 the PE cost of the native
4-cycle fp32 path. Each core:
  - replicates the GRU cell (W_hh SBUF-resident; the input-side gate terms
    gx = emb[tok] @ W_ih.T + bias are host-precomputed into a [V, 3H] table
    and fetched per step with one 32-row indirect-DMA gather),
  - computes all three gate pre-activations in ONE quadrant-packed PSUM tile
    [96, 512] (r@q0, n@q32, z@q64, double-buffered),
  - runs the GRU elementwise chain with the r-path split into two 256-wide
    halves pipelined across DVE and ACT,
  - transposes h, then splits hT into bf16 hi/lo for the scan,
  - computes logits for its 4096-entry vocab shard (col-tiled matmuls packing
    the batch=32 four-wide across PE column groups, bf16 bias quads),
  - finds per-512-chunk argmax candidates with DVE max/max_index and ships
    both halves' (val, absidx) pairs unmerged via a small AllGather,
  - selects the global winner with a value-match over all 64 candidates.
EOS masking (pad-after-done) is applied on the host: feedback uses the raw
argmax token, which provably yields identical output after masking.
"""
import numpy as np

V, E, H, B = 32000, 256, 512, 32
PAD, EOS, SOS = 0, 1, 2
N_CORES = 8
VS = 4096           # padded vocab entries per core
VPAD = VS * N_CORES  # 32768
NEG = -1.0e30


def _build(T: int):
    import concourse.bass as bass
    import concourse.bacc as bacc
    import concourse.mybir as mybir
    from concourse.tile import TileContext

    F32 = mybir.dt.float32
    BF16 = mybir.dt.bfloat16
    U32 = mybir.dt.uint32
    I32 = mybir.dt.int32
    AF = mybir.ActivationFunctionType
    OP = mybir.AluOpType

    nc = bacc.Bacc(None)

    wph_in = nc.declare_dram_parameter("wpTh", [H, VS], BF16, isOutput=False)
    wpl_in = nc.declare_dram_parameter("wpTl", [H, VS], BF16, isOutput=False)
    bph_in = nc.declare_dram_parameter("bph", [1, VS], BF16, isOutput=False)
    bpl_in = nc.declare_dram_parameter("bpl", [1, VS], BF16, isOutput=False)
    whh_in = nc.declare_dram_parameter("whhT", [H, 3 * H], F32, isOutput=False)
    bnh_in = nc.declare_dram_parameter("b_nh", [1, H], F32, isOutput=False)
    gtab_in = nc.declare_dram_parameter("gtab", [V, 3 * H], F32, isOutput=False)
    h0_in = nc.declare_dram_parameter("h0", [B, H], F32, isOutput=False)
    h0T_in = nc.declare_dram_parameter("h0T", [H, B], F32, isOutput=False)
    ident_in = nc.declare_dram_parameter("ident", [B, B], F32, isOutput=False)
    ones_in = nc.declare_dram_parameter("ones", [1, B], F32, isOutput=False)
    onesb_in = nc.declare_dram_parameter("onesb", [1, B], BF16, isOutput=False)
    pbase_in = nc.declare_dram_parameter("pbase", [128, 2], F32, isOutput=False)

    toks_out = nc.declare_dram_parameter("toks", [B, T], I32, isOutput=True)

    cc_ins = [nc.dram_tensor(f"cc_in_{t}", [128, 4], F32) for t in range(T)]
    cc_outs = [
        nc.dram_tensor(f"cc_out_{t}", [N_CORES * 128, 4], F32, addr_space="Shared")
        for t in range(T)
    ]

    HH = H // 2  # 256: the two pipelined halves of the elementwise chain

    with TileContext(nc) as tc:
        with (
            tc.tile_pool(name="wpool", bufs=1) as wpool,
            tc.tile_pool(name="state", bufs=1) as state,
            tc.tile_pool(name="sb", bufs=2) as sb,
            tc.tile_pool(name="ps_gate", bufs=2, space="PSUM") as ps_gate,
            tc.tile_pool(name="ps_tp", bufs=1, space="PSUM") as ps_tp,
            tc.tile_pool(name="ps_proj", bufs=1, space="PSUM") as ps_proj,
        ):
            # ---------- SBUF-resident weights ----------
            wph_sb, wpl_sb = [], []
            for k in range(4):
                w = wpool.tile([128, VS], BF16, tag=f"wph{k}", name=f"wph{k}")
                nc.sync.dma_start(out=w[:], in_=wph_in[128 * k:128 * (k + 1), :])
                wph_sb.append(w)
                w = wpool.tile([128, VS], BF16, tag=f"wpl{k}", name=f"wpl{k}")
                nc.sync.dma_start(out=w[:], in_=wpl_in[128 * k:128 * (k + 1), :])
                wpl_sb.append(w)
            whh_sb = []
            for k in range(4):
                w = wpool.tile([128, 3 * H], F32, tag=f"whh{k}", name=f"whh{k}")
                nc.sync.dma_start(out=w[:], in_=whh_in[128 * k:128 * (k + 1), :])
                whh_sb.append(w)
            bph_sb = wpool.tile([1, VS], BF16, tag="bph")
            nc.sync.dma_start(out=bph_sb[:], in_=bph_in[:, :])
            bpl_sb = wpool.tile([1, VS], BF16, tag="bpl")
            nc.sync.dma_start(out=bpl_sb[:], in_=bpl_in[:, :])
            bnh_sb = wpool.tile([1, H], F32, tag="bnh")
            nc.sync.dma_start(out=bnh_sb[:], in_=bnh_in[:, :])
            ident_sb = wpool.tile([B, B], F32, tag="ident")
            nc.sync.dma_start(out=ident_sb[:], in_=ident_in[:, :])
            ones_sb = wpool.tile([1, B], F32, tag="ones")
            nc.sync.dma_start(out=ones_sb[:], in_=ones_in[:, :])
            onesb_sb = wpool.tile([1, B], BF16, tag="onesb")
            nc.sync.dma_start(out=onesb_sb[:], in_=onesb_in[:, :])
            pbase_sb = wpool.tile([128, 2], F32, tag="pbase")
            nc.sync.dma_start(out=pbase_sb[:], in_=pbase_in[:, :])

            # ---------- decode state ----------
            toks_sb = state.tile([B, T], F32, tag="toks")

            h_cur = sb.tile([B, H], F32, tag="h")
            nc.sync.dma_start(out=h_cur[:], in_=h0_in[:, :])
            hT_cur = sb.tile([128, 4, B], F32, tag="hT")
            nc.sync.dma_start(
                out=hT_cur[:],
                in_=h0T_in.ap().rearrange("(k p) b -> p k b", p=128),
            )
            hThi_cur = sb.tile([128, 4, B], BF16, tag="hThi")
            nc.vector.tensor_copy(hThi_cur[:], hT_cur[:])
            hTlo_cur = sb.tile([128, 4, B], BF16, tag="hTlo")
            nc.vector.tensor_tensor(hTlo_cur[:], hT_cur[:], hThi_cur[:],
                                    op=OP.subtract)
            tok_u = sb.tile([B, 1], U32, tag="tok_u")
            nc.vector.memset(tok_u[:], SOS)

            for t in range(T):
                # ---- gate input rows: gx = gtab[tok] (= x@W_ih.T + biases) ----
                gx_sb = sb.tile([B, 3 * H], F32, tag="gx")
                nc.gpsimd.indirect_dma_start(
                    out=gx_sb[:],
                    out_offset=None,
                    in_=gtab_in[:, :],
                    in_offset=bass.IndirectOffsetOnAxis(ap=tok_u[:, :1], axis=0),
                )

                # ---- recurrent gate pre-activations:
                # r @ q0, n @ q32, z @ q64 packed in one PSUM tile
                g_rn = ps_gate.tile([96, H], F32, tag="g_rn")
                nc.tensor.matmul(g_rn[32:64, :], ones_sb[:1, :], bnh_sb[:1, :],
                                 start=True, stop=False, tile_position=(0, 32))
                for k in range(4):
                    nc.tensor.matmul(g_rn[0:32, :], hT_cur[:, k, :],
                                     whh_sb[k][:, 0:H],
                                     start=(k == 0), stop=(k == 3),
                                     tile_position=(0, 0))
                    nc.tensor.matmul(g_rn[32:64, :], hT_cur[:, k, :],
                                     whh_sb[k][:, 2 * H:3 * H],
                                     start=False, stop=(k == 3),
                                     tile_position=(0, 32))
                    nc.tensor.matmul(g_rn[64:96, :], hT_cur[:, k, :],
                                     whh_sb[k][:, H:2 * H],
                                     start=(k == 0), stop=(k == 3),
                                     tile_position=(0, 64))
                # off-critical prep (lands in the exchange window)
                ghn_sb = sb.tile([B, H], F32, tag="ghn")
                nc.vector.tensor_copy(ghn_sb[:], g_rn[32:64, :])
                gz_sb = sb.tile([B, H], F32, tag="gz_sb")
                nc.vector.tensor_copy(gz_sb[:], g_rn[64:96, :])
                h_half = sb.tile([B, H], F32, tag="h_half")
                nc.vector.tensor_scalar(h_half[:], h_cur[:], 0.5, None,
                                        op0=OP.mult)

                # ---- GRU elementwise; r-path pipelined in two 256 halves ----
                rt = sb.tile([B, H], F32, tag="rt")
                tmp = sb.tile([B, H], F32, tag="tmp")
                n_sb = sb.tile([B, H], F32, tag="n")
                h_new = sb.tile([B, H], F32, tag="h")
                rp = sb.tile([B, H], F32, tag="rp")
                zp = sb.tile([B, H], F32, tag="zp")
                zt = sb.tile([B, H], F32, tag="zt")
                omz = sb.tile([B, H], F32, tag="omz")
                zh = sb.tile([B, H], F32, tag="zh")
                for a in range(2):
                    s = slice(a * HH, (a + 1) * HH)
                    nc.vector.tensor_tensor(rp[:, s], g_rn[0:32, s],
                                            gx_sb[:, s], op=OP.add)
                    nc.scalar.activation(rt[:, s], rp[:, s], AF.Tanh, scale=0.5)
                # z pre-activation (z-pole is critical: issue right away)
                nc.vector.tensor_tensor(zp[:], gz_sb[:], gx_sb[:, H:2 * H], op=OP.add)
                nc.scalar.activation(zt[:], zp[:], AF.Tanh, scale=0.5)
                for a in range(2):
                    s = slice(a * HH, (a + 1) * HH)
                    # tmp = 0.5*((rt+1)*ghn) + gxn
                    nc.vector.scalar_tensor_tensor(tmp[:, s], rt[:, s], 1.0,
                                                   ghn_sb[:, s],
                                                   op0=OP.add, op1=OP.mult)
                    nc.vector.scalar_tensor_tensor(
                        tmp[:, s], tmp[:, s], 0.5,
                        gx_sb[:, 2 * H + a * HH:2 * H + (a + 1) * HH],
                        op0=OP.mult, op1=OP.add)
                    nc.scalar.activation(n_sb[:, s], tmp[:, s], AF.Tanh)
                for a in range(2):
                    s = slice(a * HH, (a + 1) * HH)
                    if a == 0:
                        # zh = (zt+1)*(h/2) = z*h ; omz = 0.5 - 0.5*zt = 1-z
                        nc.vector.tensor_scalar(omz[:], zt[:], -0.5, 0.5,
                                                op0=OP.mult, op1=OP.add)
                        nc.vector.scalar_tensor_tensor(zh[:], zt[:], 1.0,
                                                       h_half[:],
                                                       op0=OP.add, op1=OP.mult)
                    nc.vector.tensor_tensor(h_new[:, s], omz[:, s],
                                            n_sb[:, s], op=OP.mult)
                    nc.vector.tensor_tensor(h_new[:, s], h_new[:, s],
                                            zh[:, s], op=OP.add)

                # ---- hT (PE transpose; two psum tiles so copies pipeline) ----
                hT_psA = ps_tp.tile([128, 2, B], F32, tag="tpA")
                hT_psB = ps_tp.tile([128, 2, B], F32, tag="tpB")
                for k in range(2):
                    nc.tensor.transpose(
                        hT_psA[:, k, :], h_new[:, 128 * k:128 * (k + 1)], ident_sb[:, :]
                    )
                for k in range(2, 4):
                    nc.tensor.transpose(
                        hT_psB[:, k - 2, :], h_new[:, 128 * k:128 * (k + 1)], ident_sb[:, :]
                    )
                hT_new = sb.tile([128, 4, B], F32, tag="hT")
                nc.vector.tensor_copy(hT_new[:, 0:2, :], hT_psA[:])
                nc.vector.tensor_copy(hT_new[:, 2:4, :], hT_psB[:])
                # bf16 hi/lo split of hT for the scan
                hThi = sb.tile([128, 4, B], BF16, tag="hThi")
                nc.vector.tensor_copy(hThi[:], hT_new[:])
                hTlo = sb.tile([128, 4, B], BF16, tag="hTlo")
                nc.vector.tensor_tensor(hTlo[:], hT_new[:], hThi[:], op=OP.subtract)

                # ---- projection scan: 3 bf16 passes per k-chunk ----
                pjs = [ps_proj.tile([128, 512], F32, tag="proj0", name="pj0"),
                       ps_proj.tile([128, 512], F32, tag="proj1", name="pj1")]
                for tt in range(2):
                    pj = pjs[tt]
                    for g in range(4):
                        sl = slice(g * 1024 + tt * 512, g * 1024 + tt * 512 + 512)
                        nc.tensor.matmul(
                            pj[32 * g:32 * (g + 1), :], onesb_sb[:1, :], bph_sb[:1, sl],
                            start=True, stop=False, tile_position=(0, 32 * g))
                        nc.tensor.matmul(
                            pj[32 * g:32 * (g + 1), :], onesb_sb[:1, :], bpl_sb[:1, sl],
                            start=False, stop=False, tile_position=(0, 32 * g))
                    for k in range(4):
                        for hi_t, w_t, last in ((hThi, wph_sb, False),
                                                (hTlo, wph_sb, False),
                                                (hThi, wpl_sb, True)):
                            for g in range(4):
                                sl = slice(g * 1024 + tt * 512,
                                           g * 1024 + tt * 512 + 512)
                                nc.tensor.matmul(
                                    pj[32 * g:32 * (g + 1), :],
                                    hi_t[:, k, :],
                                    w_t[k][:, sl],
                                    start=False, stop=(k == 3 and last),
                                    tile_position=(0, 32 * g))

                # ---- per-half candidates: (val, absidx) straight into cand ----
                cand = sb.tile([128, 4], F32, tag="cand")
                for tt in range(2):
                    mx = sb.tile([128, 8], F32, tag=f"mx{tt}", name=f"mx{tt}")
                    mi = sb.tile([128, 8], U32, tag=f"mi{tt}", name=f"mi{tt}")
                    nc.vector.max(out=mx[:], in_=pjs[tt][:, :])
                    nc.vector.max_index(mi[:], mx[:], pjs[tt][:, :])
                    nc.vector.tensor_copy(cand[:, 2 * tt:2 * tt + 1], mx[:, 0:1])
                    nc.vector.tensor_copy(cand[:, 2 * tt + 1:2 * tt + 2], mi[:, 0:1])
                    nc.vector.tensor_tensor(cand[:, 2 * tt + 1:2 * tt + 2],
                                            cand[:, 2 * tt + 1:2 * tt + 2],
                                            pbase_sb[:, tt:tt + 1], op=OP.add)

                # ---- exchange across cores ----
                nc.sync.dma_start(out=cc_ins[t][:, :], in_=cand[:])
                nc.gpsimd.collective_compute(
                    "AllGather",
                    mybir.AluOpType.bypass,
                    replica_groups=[list(range(N_CORES))],
                    ins=[cc_ins[t].ap().opt()],
                    outs=[cc_outs[t].ap().opt()],
                )
                # gath[b, rg, h, c]: c=0 value, c=1 absolute index
                gath = sb.tile([B, 32, 2, 2], F32, tag="gath")
                nc.sync.dma_start(
                    out=gath[:],
                    in_=cc_outs[t].ap().rearrange("(r g b) (h c) -> b (r g) h c",
                                                  r=8, g=4, h=2),
                )

                # ---- global winner: max value, then index by value-match ----
                wmax = sb.tile([B, 1], F32, tag="wmax")
                nc.vector.tensor_reduce(wmax[:, 0:1], gath[:, :, :, 0:1],
                                        axis=mybir.AxisListType.XYZ, op=OP.max)
                weq = sb.tile([B, 32, 2], F32, tag="weq")
                nc.vector.tensor_scalar(weq[:], gath[:, :, :, 0:1], wmax[:, 0:1],
                                        None, op0=OP.is_equal)
                nc.vector.tensor_tensor(weq[:], weq[:], gath[:, :, :, 1:2],
                                        op=OP.mult)
                # raw argmax token straight into the output row + uint copy
                nc.vector.tensor_reduce(toks_sb[:, t:t + 1], weq[:],
                                        axis=mybir.AxisListType.XY, op=OP.max)
                tok_u = sb.tile([B, 1], U32, tag="tok_u")
                nc.vector.tensor_copy(tok_u[:], toks_sb[:, t:t + 1])

                h_cur = h_new
                hT_cur = hT_new
                hThi_cur = hThi
                hTlo_cur = hTlo

            toks_i = state.tile([B, T], I32, tag="toks_i")
            nc.vector.tensor_copy(toks_i[:], toks_sb[:])
            nc.sync.dma_start(out=toks_out[:, :], in_=toks_i[:])

    nc.compile()
    return nc


_NC_CACHE = {}
TRACE = False
LAST_EXEC_NS = None


def _to_bf16_np(x):
    xi = np.ascontiguousarray(x, dtype=np.float32).view(np.uint32)
    xr = ((xi + 0x7FFF + ((xi >> 16) & 1)) & 0xFFFF0000).view(np.float32)
    return xr


def kernel(hidden, emb, W_ih, W_hh, b_ih, b_hh, W_proj, b_proj, max_len, **_):
    from concourse.bass_utils import run_bass_kernel_spmd
    import ml_dtypes

    T = int(max_len)
    hidden = np.asarray(hidden, dtype=np.float32)
    emb = np.asarray(emb, dtype=np.float32)
    W_ih = np.asarray(W_ih, dtype=np.float32)
    W_hh = np.asarray(W_hh, dtype=np.float32)
    b_ih = np.asarray(b_ih, dtype=np.float32)
    b_hh = np.asarray(b_hh, dtype=np.float32)
    W_proj = np.asarray(W_proj, dtype=np.float32)
    b_proj = np.asarray(b_proj, dtype=np.float32)

    # input-side gate table: gtab[v] = emb[v] @ W_ih.T (+ r,z biases / x-side n bias)
    gtab = emb @ np.ascontiguousarray(W_ih.T)
    gtab[:, 0:2 * H] += (b_ih + b_hh)[None, 0:2 * H]
    gtab[:, 2 * H:3 * H] += b_ih[None, 2 * H:3 * H]
    gtab = np.ascontiguousarray(gtab, dtype=np.float32)

    # pad vocab so every core owns exactly VS rows; padded logits = -1e30
    Wp = np.zeros((VPAD, H), dtype=np.float32)
    Wp[:V] = W_proj
    bp = np.full((VPAD,), NEG, dtype=np.float32)
    bp[:V] = b_proj

    # bf16 hi/lo splits of the projection weight and bias
    Wp_hi = _to_bf16_np(Wp)
    Wp_lo = _to_bf16_np(Wp - Wp_hi)
    bp_hi = _to_bf16_np(bp)
    bp_lo = _to_bf16_np(bp - bp_hi)

    whhT = np.ascontiguousarray(W_hh.T)
    b_nh = np.ascontiguousarray(b_hh[None, 2 * H:3 * H])
    h0 = np.ascontiguousarray(hidden[0])
    h0T = np.ascontiguousarray(h0.T)
    ident = np.eye(B, dtype=np.float32)
    ones = np.ones((1, B), dtype=np.float32)
    onesb = np.ones((1, B), dtype=ml_dtypes.bfloat16)

    if T not in _NC_CACHE:
        _NC_CACHE[T] = _build(T)
    nc = _NC_CACHE[T]

    in_maps = []
    for c in range(N_CORES):
        # pbase[p, tt]: absolute vocab base of (group g = p//32, half tt)
        g = (np.arange(128) // 32)
        pbase = np.stack([c * VS + g * 1024, c * VS + g * 1024 + 512],
                         axis=1).astype(np.float32)
        sl = slice(c * VS, (c + 1) * VS)
        in_maps.append({
            "wpTh": np.ascontiguousarray(Wp_hi[sl].T).astype(ml_dtypes.bfloat16),
            "wpTl": np.ascontiguousarray(Wp_lo[sl].T).astype(ml_dtypes.bfloat16),
            "bph": np.ascontiguousarray(bp_hi[None, sl]).astype(ml_dtypes.bfloat16),
            "bpl": np.ascontiguousarray(bp_lo[None, sl]).astype(ml_dtypes.bfloat16),
            "whhT": whhT,
            "b_nh": b_nh,
            "gtab": gtab,
            "h0": h0, "h0T": h0T,
            "ident": ident, "ones": ones, "onesb": onesb,
            "pbase": pbase,
        })

    global LAST_EXEC_NS
    res = run_bass_kernel_spmd(nc, in_maps, core_ids=list(range(N_CORES)), trace=TRACE)
    LAST_EXEC_NS = res.exec_time_ns
    toks = res.results[0]["toks"]
    raw = np.ascontiguousarray(toks.T.astype(np.int32))  # [T, B] raw argmax tokens

    # host-side EOS masking: once a row emits EOS, all later tokens become EOS
    out = raw.copy()
    for b in range(B):
        eos_t = np.where(raw[:, b] == EOS)[0]
        if len(eos_t):
            out[eos_t[0] + 1:, b] = EOS
    return out


# revision 4
# speedup vs baseline: 1.0740x; 1.0740x over previous
"""Batched greedy GRU decoder on 8 Trainium2 NeuronCores.

Strategy: tensor-parallel over the vocabulary. W_proj [32000,512] fp32 (65.5MB)
cannot fit in one core's 28MB SBUF, but an 8-way shard (padded to 4096
rows/core, 8MB) stays SBUF-resident across all 64 decode steps. Each core:
  - replicates the GRU cell (W_hh SBUF-resident; the input-side gate terms
    gx = emb[tok] @ W_ih.T + bias are host-precomputed into a [V, 3H] table
    and fetched per step with one 32-row indirect-DMA gather),
  - computes all three gate pre-activations in ONE quadrant-packed PSUM tile
    [96, 512] (r@q0, n@q32, z@q64, double-buffered) so the recurrent matmuls
    take 1/3 the PE wall-clock and hide under the previous step's exchange,
  - runs the GRU elementwise chain with the r-path split into two 256-wide
    halves pipelined across DVE and ACT (ghn/gz copied off PSUM in the
    exchange window; zh folded to one fused op via a precomputed h/2),
  - computes logits for its 4096-entry vocab shard (col-tiled matmuls packing
    the batch=32 four-wide across PE column groups),
  - finds per-512-chunk argmax candidates with DVE max/max_index and ships
    both halves' (val, absidx) pairs unmerged via a small AllGather,
  - selects the global winner with a value-match over all 64 candidates.
Output tokens accumulate in SBUF and are written out once at the end.
"""
import numpy as np

V, E, H, B = 32000, 256, 512, 32
PAD, EOS, SOS = 0, 1, 2
N_CORES = 8
VS = 4096           # padded vocab entries per core
VPAD = VS * N_CORES  # 32768
NEG = -1.0e30


def _build(T: int):
    import concourse.bass as bass
    import concourse.bacc as bacc
    import concourse.mybir as mybir
    from concourse.tile import TileContext

    F32 = mybir.dt.float32
    U32 = mybir.dt.uint32
    I32 = mybir.dt.int32
    AF = mybir.ActivationFunctionType
    OP = mybir.AluOpType

    nc = bacc.Bacc(None)

    wproj_in = nc.declare_dram_parameter("wprojT", [H, VS], F32, isOutput=False)
    bproj_in = nc.declare_dram_parameter("bproj", [1, VS], F32, isOutput=False)
    whh_in = nc.declare_dram_parameter("whhT", [H, 3 * H], F32, isOutput=False)
    bnh_in = nc.declare_dram_parameter("b_nh", [1, H], F32, isOutput=False)
    gtab_in = nc.declare_dram_parameter("gtab", [V, 3 * H], F32, isOutput=False)
    h0_in = nc.declare_dram_parameter("h0", [B, H], F32, isOutput=False)
    h0T_in = nc.declare_dram_parameter("h0T", [H, B], F32, isOutput=False)
    ident_in = nc.declare_dram_parameter("ident", [B, B], F32, isOutput=False)
    ones_in = nc.declare_dram_parameter("ones", [1, B], F32, isOutput=False)
    pbase_in = nc.declare_dram_parameter("pbase", [128, 2], F32, isOutput=False)

    toks_out = nc.declare_dram_parameter("toks", [B, T], I32, isOutput=True)

    cc_ins = [nc.dram_tensor(f"cc_in_{t}", [128, 4], F32) for t in range(T)]
    cc_outs = [
        nc.dram_tensor(f"cc_out_{t}", [N_CORES * 128, 4], F32, addr_space="Shared")
        for t in range(T)
    ]

    HH = H // 2  # 256: the two pipelined halves of the elementwise chain

    with TileContext(nc) as tc:
        with (
            tc.tile_pool(name="wpool", bufs=1) as wpool,
            tc.tile_pool(name="state", bufs=1) as state,
            tc.tile_pool(name="sb", bufs=2) as sb,
            tc.tile_pool(name="ps_gate", bufs=2, space="PSUM") as ps_gate,
            tc.tile_pool(name="ps_tp", bufs=1, space="PSUM") as ps_tp,
            tc.tile_pool(name="ps_proj", bufs=1, space="PSUM") as ps_proj,
        ):
            # ---------- SBUF-resident weights ----------
            wp_sb = []
            for k in range(4):
                w = wpool.tile([128, VS], F32, tag=f"wp{k}", name=f"wp{k}")
                nc.sync.dma_start(out=w[:], in_=wproj_in[128 * k:128 * (k + 1), :])
                wp_sb.append(w)
            whh_sb = []
            for k in range(4):
                w = wpool.tile([128, 3 * H], F32, tag=f"whh{k}", name=f"whh{k}")
                nc.sync.dma_start(out=w[:], in_=whh_in[128 * k:128 * (k + 1), :])
                whh_sb.append(w)
            bp_sb = wpool.tile([1, VS], F32, tag="bp")
            nc.sync.dma_start(out=bp_sb[:], in_=bproj_in[:, :])
            bnh_sb = wpool.tile([1, H], F32, tag="bnh")
            nc.sync.dma_start(out=bnh_sb[:], in_=bnh_in[:, :])
            ident_sb = wpool.tile([B, B], F32, tag="ident")
            nc.sync.dma_start(out=ident_sb[:], in_=ident_in[:, :])
            ones_sb = wpool.tile([1, B], F32, tag="ones")
            nc.sync.dma_start(out=ones_sb[:], in_=ones_in[:, :])
            pbase_sb = wpool.tile([128, 2], F32, tag="pbase")
            nc.sync.dma_start(out=pbase_sb[:], in_=pbase_in[:, :])

            # ---------- decode state ----------
            toks_sb = state.tile([B, T], F32, tag="toks")

            h_cur = sb.tile([B, H], F32, tag="h")
            nc.sync.dma_start(out=h_cur[:], in_=h0_in[:, :])
            hT_cur = sb.tile([128, 4, B], F32, tag="hT")
            nc.sync.dma_start(
                out=hT_cur[:],
                in_=h0T_in.ap().rearrange("(k p) b -> p k b", p=128),
            )
            tok_u = sb.tile([B, 1], U32, tag="tok_u")
            nc.vector.memset(tok_u[:], SOS)

            for t in range(T):
                # ---- gate input rows: gx = gtab[tok] (= x@W_ih.T + biases) ----
                gx_sb = sb.tile([B, 3 * H], F32, tag="gx")
                nc.gpsimd.indirect_dma_start(
                    out=gx_sb[:],
                    out_offset=None,
                    in_=gtab_in[:, :],
                    in_offset=bass.IndirectOffsetOnAxis(ap=tok_u[:, :1], axis=0),
                )

                # ---- recurrent gate pre-activations:
                # r @ q0 and n @ q32 packed in one tile; z @ q0 separately
                # (z stays on partitions 0:32 so its branch needs no moves;
                # only ghn is copied down, off the critical path)
                g_rn = ps_gate.tile([96, H], F32, tag="g_rn")
                nc.tensor.matmul(g_rn[32:64, :], ones_sb[:1, :], bnh_sb[:1, :],
                                 start=True, stop=False, tile_position=(0, 32))
                for k in range(4):
                    nc.tensor.matmul(g_rn[0:32, :], hT_cur[:, k, :],
                                     whh_sb[k][:, 0:H],
                                     start=(k == 0), stop=(k == 3),
                                     tile_position=(0, 0))
                    nc.tensor.matmul(g_rn[32:64, :], hT_cur[:, k, :],
                                     whh_sb[k][:, 2 * H:3 * H],
                                     start=False, stop=(k == 3),
                                     tile_position=(0, 32))
                    nc.tensor.matmul(g_rn[64:96, :], hT_cur[:, k, :],
                                     whh_sb[k][:, H:2 * H],
                                     start=(k == 0), stop=(k == 3),
                                     tile_position=(0, 64))
                # off-critical prep (lands in the exchange window)
                ghn_sb = sb.tile([B, H], F32, tag="ghn")
                nc.vector.tensor_copy(ghn_sb[:], g_rn[32:64, :])
                gz_sb = sb.tile([B, H], F32, tag="gz_sb")
                nc.vector.tensor_copy(gz_sb[:], g_rn[64:96, :])
                h_half = sb.tile([B, H], F32, tag="h_half")
                nc.vector.tensor_scalar(h_half[:], h_cur[:], 0.5, None,
                                        op0=OP.mult)

                # ---- GRU elementwise; r-path pipelined in two 256 halves ----
                rt = sb.tile([B, H], F32, tag="rt")
                tmp = sb.tile([B, H], F32, tag="tmp")
                n_sb = sb.tile([B, H], F32, tag="n")
                h_new = sb.tile([B, H], F32, tag="h")
                rp = sb.tile([B, H], F32, tag="rp")
                zp = sb.tile([B, H], F32, tag="zp")
                zt = sb.tile([B, H], F32, tag="zt")
                omz = sb.tile([B, H], F32, tag="omz")
                zh = sb.tile([B, H], F32, tag="zh")
                for a in range(2):
                    s = slice(a * HH, (a + 1) * HH)
                    nc.vector.tensor_tensor(rp[:, s], g_rn[0:32, s],
                                            gx_sb[:, s], op=OP.add)
                    nc.scalar.activation(rt[:, s], rp[:, s], AF.Tanh, scale=0.5)
                # z pre-activation (z-pole is critical: issue right away)
                nc.vector.tensor_tensor(zp[:], gz_sb[:], gx_sb[:, H:2 * H], op=OP.add)
                nc.scalar.activation(zt[:], zp[:], AF.Tanh, scale=0.5)
                for a in range(2):
                    s = slice(a * HH, (a + 1) * HH)
                    # tmp = 0.5*((rt+1)*ghn) + gxn
                    nc.vector.scalar_tensor_tensor(tmp[:, s], rt[:, s], 1.0,
                                                   ghn_sb[:, s],
                                                   op0=OP.add, op1=OP.mult)
                    nc.vector.scalar_tensor_tensor(
                        tmp[:, s], tmp[:, s], 0.5,
                        gx_sb[:, 2 * H + a * HH:2 * H + (a + 1) * HH],
                        op0=OP.mult, op1=OP.add)
                    nc.scalar.activation(n_sb[:, s], tmp[:, s], AF.Tanh)
                for a in range(2):
                    s = slice(a * HH, (a + 1) * HH)
                    if a == 0:
                        # zh = (zt+1)*(h/2) = z*h ; omz = 0.5 - 0.5*zt = 1-z
                        nc.vector.tensor_scalar(omz[:], zt[:], -0.5, 0.5,
                                                op0=OP.mult, op1=OP.add)
                        nc.vector.scalar_tensor_tensor(zh[:], zt[:], 1.0,
                                                       h_half[:],
                                                       op0=OP.add, op1=OP.mult)
                    nc.vector.tensor_tensor(h_new[:, s], omz[:, s],
                                            n_sb[:, s], op=OP.mult)
                    nc.vector.tensor_tensor(h_new[:, s], h_new[:, s],
                                            zh[:, s], op=OP.add)

                # ---- hT (PE transpose; two psum tiles so copies pipeline) ----
                hT_psA = ps_tp.tile([128, 2, B], F32, tag="tpA")
                hT_psB = ps_tp.tile([128, 2, B], F32, tag="tpB")
                for k in range(2):
                    nc.tensor.transpose(
                        hT_psA[:, k, :], h_new[:, 128 * k:128 * (k + 1)], ident_sb[:, :]
                    )
                for k in range(2, 4):
                    nc.tensor.transpose(
                        hT_psB[:, k - 2, :], h_new[:, 128 * k:128 * (k + 1)], ident_sb[:, :]
                    )
                hT_new = sb.tile([128, 4, B], F32, tag="hT")
                nc.vector.tensor_copy(hT_new[:, 0:2, :], hT_psA[:])
                nc.vector.tensor_copy(hT_new[:, 2:4, :], hT_psB[:])

                # ---- projection (vocab of group g, half tt: g*1024 + tt*512 + f) ----
                pjs = [ps_proj.tile([128, 512], F32, tag="proj0", name="pj0"),
                       ps_proj.tile([128, 512], F32, tag="proj1", name="pj1")]
                for tt in range(2):
                    pj = pjs[tt]
                    for g in range(4):
                        nc.tensor.matmul(
                            pj[32 * g:32 * (g + 1), :],
                            ones_sb[:1, :],
                            bp_sb[:1, g * 1024 + tt * 512:g * 1024 + tt * 512 + 512],
                            start=True, stop=False,
                            tile_position=(0, 32 * g),
                        )
                    for k in range(4):
                        for g in range(4):
                            nc.tensor.matmul(
                                pj[32 * g:32 * (g + 1), :],
                                hT_new[:, k, :],
                                wp_sb[k][:, g * 1024 + tt * 512:g * 1024 + tt * 512 + 512],
                                start=False, stop=(k == 3),
                                tile_position=(0, 32 * g),
                            )

                # ---- per-half candidates: (val, absidx) straight into cand ----
                cand = sb.tile([128, 4], F32, tag="cand")
                for tt in range(2):
                    mx = sb.tile([128, 8], F32, tag=f"mx{tt}", name=f"mx{tt}")
                    mi = sb.tile([128, 8], U32, tag=f"mi{tt}", name=f"mi{tt}")
                    nc.vector.max(out=mx[:], in_=pjs[tt][:, :])
                    nc.vector.max_index(mi[:], mx[:], pjs[tt][:, :])
                    nc.vector.tensor_copy(cand[:, 2 * tt:2 * tt + 1], mx[:, 0:1])
                    nc.vector.tensor_copy(cand[:, 2 * tt + 1:2 * tt + 2], mi[:, 0:1])
                    nc.vector.tensor_tensor(cand[:, 2 * tt + 1:2 * tt + 2],
                                            cand[:, 2 * tt + 1:2 * tt + 2],
                                            pbase_sb[:, tt:tt + 1], op=OP.add)

                # ---- exchange across cores ----
                nc.sync.dma_start(out=cc_ins[t][:, :], in_=cand[:])
                nc.gpsimd.collective_compute(
                    "AllGather",
                    mybir.AluOpType.bypass,
                    replica_groups=[list(range(N_CORES))],
                    ins=[cc_ins[t].ap().opt()],
                    outs=[cc_outs[t].ap().opt()],
                )
                # gath[b, rg, h, c]: c=0 value, c=1 absolute index
                gath = sb.tile([B, 32, 2, 2], F32, tag="gath")
                nc.sync.dma_start(
                    out=gath[:],
                    in_=cc_outs[t].ap().rearrange("(r g b) (h c) -> b (r g) h c",
                                                  r=8, g=4, h=2),
                )

                # ---- global winner: max value, then index by value-match ----
                wmax = sb.tile([B, 1], F32, tag="wmax")
                nc.vector.tensor_reduce(wmax[:, 0:1], gath[:, :, :, 0:1],
                                        axis=mybir.AxisListType.XYZ, op=OP.max)
                weq = sb.tile([B, 32, 2], F32, tag="weq")
                nc.vector.tensor_scalar(weq[:], gath[:, :, :, 0:1], wmax[:, 0:1],
                                        None, op0=OP.is_equal)
                nc.vector.tensor_tensor(weq[:], weq[:], gath[:, :, :, 1:2],
                                        op=OP.mult)
                # raw argmax token straight into the output row + uint copy
                nc.vector.tensor_reduce(toks_sb[:, t:t + 1], weq[:],
                                        axis=mybir.AxisListType.XY, op=OP.max)
                tok_u = sb.tile([B, 1], U32, tag="tok_u")
                nc.vector.tensor_copy(tok_u[:], toks_sb[:, t:t + 1])
                h_cur = h_new
                hT_cur = hT_new

            toks_i = state.tile([B, T], I32, tag="toks_i")
            nc.vector.tensor_copy(toks_i[:], toks_sb[:])
            nc.sync.dma_start(out=toks_out[:, :], in_=toks_i[:])

    nc.compile()
    return nc


_NC_CACHE = {}
TRACE = False
LAST_EXEC_NS = None


def kernel(hidden, emb, W_ih, W_hh, b_ih, b_hh, W_proj, b_proj, max_len, **_):
    from concourse.bass_utils import run_bass_kernel_spmd

    T = int(max_len)
    hidden = np.asarray(hidden, dtype=np.float32)
    emb = np.asarray(emb, dtype=np.float32)
    W_ih = np.asarray(W_ih, dtype=np.float32)
    W_hh = np.asarray(W_hh, dtype=np.float32)
    b_ih = np.asarray(b_ih, dtype=np.float32)
    b_hh = np.asarray(b_hh, dtype=np.float32)
    W_proj = np.asarray(W_proj, dtype=np.float32)
    b_proj = np.asarray(b_proj, dtype=np.float32)

    # input-side gate table: gtab[v] = emb[v] @ W_ih.T (+ r,z biases / x-side n bias)
    gtab = emb @ np.ascontiguousarray(W_ih.T)
    gtab[:, 0:2 * H] += (b_ih + b_hh)[None, 0:2 * H]
    gtab[:, 2 * H:3 * H] += b_ih[None, 2 * H:3 * H]
    gtab = np.ascontiguousarray(gtab, dtype=np.float32)

    # pad vocab so every core owns exactly VS rows; padded logits = -1e30
    Wp = np.zeros((VPAD, H), dtype=np.float32)
    Wp[:V] = W_proj
    bp = np.full((VPAD,), NEG, dtype=np.float32)
    bp[:V] = b_proj

    whhT = np.ascontiguousarray(W_hh.T)
    b_nh = np.ascontiguousarray(b_hh[None, 2 * H:3 * H])
    h0 = np.ascontiguousarray(hidden[0])
    h0T = np.ascontiguousarray(h0.T)
    ident = np.eye(B, dtype=np.float32)
    ones = np.ones((1, B), dtype=np.float32)

    if T not in _NC_CACHE:
        _NC_CACHE[T] = _build(T)
    nc = _NC_CACHE[T]

    in_maps = []
    for c in range(N_CORES):
        # pbase[p, tt]: absolute vocab base of (group g = p//32, half tt)
        g = (np.arange(128) // 32)
        pbase = np.stack([c * VS + g * 1024, c * VS + g * 1024 + 512],
                         axis=1).astype(np.float32)
        in_maps.append({
            "wprojT": np.ascontiguousarray(Wp[c * VS:(c + 1) * VS].T),
            "bproj": np.ascontiguousarray(bp[None, c * VS:(c + 1) * VS]),
            "whhT": whhT,
            "b_nh": b_nh,
            "gtab": gtab,
            "h0": h0, "h0T": h0T,
            "ident": ident, "ones": ones,
            "pbase": pbase,
        })

    global LAST_EXEC_NS
    res = run_bass_kernel_spmd(nc, in_maps, core_ids=list(range(N_CORES)), trace=TRACE)
    LAST_EXEC_NS = res.exec_time_ns
    toks = res.results[0]["toks"]
    raw = np.ascontiguousarray(toks.T.astype(np.int32))  # [T, B] raw argmax tokens

    # host-side EOS masking: once a row emits EOS, all later tokens become EOS.
    # Equivalent to the reference's in-loop masking: batch rows are independent,
    # so a done row's h trajectory cannot affect any other row, and its own
    # outputs are EOS regardless of what is fed back.
    out = raw.copy()
    for b in range(B):
        eos_t = np.where(raw[:, b] == EOS)[0]
        if len(eos_t):
            out[eos_t[0] + 1:, b] = EOS
    return out

